# revision 39
# baseline (speedup 1.0000x reference)
"""Trainium2 Bass kernel for nn_AELoss (MSE + smooth loss), 8-core data-parallel.

Strategy
--------
Shard batch dim (2048) across 8 cores -> 256 rows/core. Per core, per
(b-group of 128, c) step, DMA-load inputs+targets as ONE [128, 2, 300, 25]
bf16 tile (SWDGE cast f32->bf16 in the DMA, so HBM reads stay f32 but all
on-chip compute runs in bf16 / 2x DVE mode).

Math (per b, c, j):  with A = sum_t x[t], Q = sum_t x[t]^2:
    s_x = A - x[T-1] - Q + x[0]^2   (= sum_{t<T-1} x[t] - x[t+1]^2)
    total[b,c] = sum_{j<J-1} |s_in - s_tgt|
    smooth partial = sum_{b,c} sqrt(total) / (J*T)
    mse partial    = sum x^2 + sum y^2 - 2*sum x*y  (reuses Q sums + one
                     scalar_tensor_tensor pass with accum for the cross term)

Engines: DVE does fold-trees over t (bf16 tensor_tensor at 2x) and the
cross-term pass; ACT does the squares; gpsimd issues cast-DMAs and the final
partition reduction. Per-core partial sums are returned as a [1,2] tensor;
the host combines the 8 cores' partials into the final scalar.
"""

import os
import sys

for _p in ("/opt/trn_rl_repo", "/root/.axon_site"):
    if os.path.isdir(_p) and _p not in sys.path:
        sys.path.insert(0, _p)

import numpy as np

import concourse.bass as bass
import concourse.tile as tile
from concourse import bacc, bass_isa, mybir
from concourse.bass_utils import run_bass_kernel_spmd

N_CORES = 8
B, C, T, J = 2048, 3, 300, 25
B_LOC = B // N_CORES          # 256 batch rows per core
P = 128                       # SBUF partitions
NG = B_LOC // P               # 2 b-groups per core
F32 = mybir.dt.float32
BF16 = mybir.dt.bfloat16
NSTEP = NG * C                # 6 (b-group, c) steps


def _fold_t2(nc, fs_pool, src, res):
    """Sum src [P, 2, 300, 25] over the t axis -> res [P, 2, 25] f32.

    Binary fold tree in bf16: 300 = 2*128 + 44, halve down to 2 rows,
    final add writes f32.
    """
    v = nc.vector
    fs = fs_pool.tile([P, 2, 128, J], BF16, tag="fold_bf")
    v.tensor_add(fs[:, :, 0:128, :], src[:, :, 0:128, :], src[:, :, 128:256, :])
    v.tensor_add(fs[:, :, 0:44, :], fs[:, :, 0:44, :], src[:, :, 256:300, :])
    n = 64
    while n >= 2:
        v.tensor_add(fs[:, :, 0:n, :], fs[:, :, 0:n, :], fs[:, :, n : 2 * n, :])
        n //= 2
    v.tensor_add(res[:, :, :], fs[:, :, 0, :], fs[:, :, 1, :])


def _body(tc, nc, x_d, y_d, out_d):
    sub = mybir.AluOpType.subtract
    add = mybir.AluOpType.add
    mult = mybir.AluOpType.mult
    bypass = mybir.AluOpType.bypass

    TH = T // 2  # 150, t-half for DMA/elementwise pipelining

    with (
        tc.tile_pool(name="inp", bufs=4) as inp_pool,
        tc.tile_pool(name="sd", bufs=3) as sd_pool,
        tc.tile_pool(name="wp", bufs=2) as w_pool,
        tc.tile_pool(name="fold", bufs=2) as fold_pool,
        tc.tile_pool(name="small", bufs=3) as small_pool,
        tc.tile_pool(name="persist", bufs=1) as persist,
    ):
        totals6 = persist.tile([P, NSTEP], F32)       # per-step sum_j |s_in - s_tgt|
        mse14 = persist.tile([P, 2 * NSTEP + 2], F32)  # per-chunk sum (x-y)^2

        k = 0
        mcol = 0
        for g in range(NG):
            for c in range(C):
                # sd[:,0] = s = x+y (-> becomes p = x^2-y^2), sd[:,1] = d = x-y
                sd = sd_pool.tile([P, 2, T, J], BF16, tag="sd")
                # first step uses quarter chunks so compute starts sooner
                nch = 4 if k == 0 else 2
                tc_sz = T // nch
                for h in range(nch):
                    t0, t1 = h * tc_sz, (h + 1) * tc_sz
                    xyh = inp_pool.tile([P, 2, tc_sz, J], BF16, tag="xy")
                    nc.gpsimd.dma_start(
                        out=xyh[:, 0, :, :],
                        in_=x_d[g * P : (g + 1) * P, c, t0:t1, :],
                    )
                    nc.gpsimd.dma_start(
                        out=xyh[:, 1, :, :],
                        in_=y_d[g * P : (g + 1) * P, c, t0:t1, :],
                    )
                    nc.vector.tensor_add(
                        sd[:, 0, t0:t1, :], xyh[:, 0, :, :], xyh[:, 1, :, :]
                    )
                    nc.vector.tensor_sub(
                        sd[:, 1, t0:t1, :], xyh[:, 0, :, :], xyh[:, 1, :, :]
                    )
                    # p = s*d = x^2-y^2, in place over s
                    nc.vector.tensor_mul(
                        sd[:, 0, t0:t1, :], sd[:, 0, t0:t1, :], sd[:, 1, t0:t1, :]
                    )
                    # MSE partial for this chunk: sum d^2 (ACT square with
                    # accumulate; junk elementwise output goes to the xyh
                    # tile we just consumed)
                    nc.scalar.activation(
                        xyh[:, 0, :, :],
                        sd[:, 1, t0:t1, :],
                        mybir.ActivationFunctionType.Square,
                        accum_out=mse14[:, mcol : mcol + 1],
                    )
                    mcol += 1

                # one combined fold chain: res[:,0]=Pd=sum_t p, res[:,1]=Ad=sum_t d
                res = small_pool.tile([P, 2, J], F32, tag="res")
                _fold_t2(nc, fold_pool, sd, res)

                # D[j] = s_in - s_tgt = Ad - Pd + p[0] - d[T-1]
                D = small_pool.tile([P, J], F32, tag="D")
                nc.vector.tensor_sub(D[:, :], res[:, 1, :], res[:, 0, :])
                nc.vector.tensor_add(D[:, :], D[:, :], sd[:, 0, 0, :])
                nc.vector.tensor_sub(D[:, :], D[:, :], sd[:, 1, T - 1, :])
                nc.vector.reduce_sum(
                    totals6[:, k : k + 1],
                    D[:, 0 : J - 1],
                    axis=mybir.AxisListType.X,
                    apply_absolute_value=True,
                )

                k += 1

        # tail: sqrt(total)/(J*T) == sqrt(total * (1/(J*T))^2), summed over steps
        roots = small_pool.tile([P, NSTEP], F32, tag="roots")
        nc.scalar.activation(
            roots[:, :],
            totals6[:, :],
            mybir.ActivationFunctionType.Sqrt,
            scale=1.0 / float((J * T) ** 2),
        )
        final = small_pool.tile([P, 2], F32, tag="final")
        nc.vector.reduce_sum(final[:, 1:2], roots[:, :], axis=mybir.AxisListType.X)
        nc.vector.reduce_sum(final[:, 0:1], mse14[:, :], axis=mybir.AxisListType.X)

        red = small_pool.tile([P, 2], F32, tag="red")
        nc.gpsimd.partition_all_reduce(
            red[:, :], final[:, :], channels=P, reduce_op=bass_isa.ReduceOp.add
        )
        nc.sync.dma_start(out=out_d[0:1, :], in_=red[0:1, :])


_NC_CACHE = None


def _build():
    global _NC_CACHE
    if _NC_CACHE is not None:
        return _NC_CACHE
    nc = bacc.Bacc("TRN2", target_bir_lowering=False, debug=False, num_devices=N_CORES)
    x_d = nc.dram_tensor("inputs", [B_LOC, C, T, J], F32, kind="ExternalInput")
    y_d = nc.dram_tensor("targets", [B_LOC, C, T, J], F32, kind="ExternalInput")
    out_d = nc.dram_tensor("out", [1, 2], F32, kind="ExternalOutput")
    with tile.TileContext(nc) as tc:
        _body(tc, nc, x_d.ap(), y_d.ap(), out_d.ap())
    nc.compile()
    _NC_CACHE = nc
    return nc


def _run(inputs, targets, trace=False, **kw):
    nc = _build()
    inputs = np.ascontiguousarray(inputs, dtype=np.float32)
    targets = np.ascontiguousarray(targets, dtype=np.float32)
    in_maps = [
        {
            "inputs": inputs[i * B_LOC : (i + 1) * B_LOC],
            "targets": targets[i * B_LOC : (i + 1) * B_LOC],
        }
        for i in range(N_CORES)
    ]
    res = run_bass_kernel_spmd(
        nc, in_maps, core_ids=list(range(N_CORES)), trace=trace, **kw
    )
    mse_sum = 0.0
    smooth_sum = 0.0
    for i in range(N_CORES):
        o = res.results[i]["out"]
        mse_sum += float(o[0, 0])
        smooth_sum += float(o[0, 1])
    value = 2.0 * (mse_sum / (B * C * T * J)) + 3.0 * (smooth_sum / (B * C))
    return np.array(value, dtype=np.float32), res


def kernel(inputs, targets):
    value, _ = _run(inputs, targets)
    return value


# revision 43
# speedup vs baseline: 1.1746x; 1.1746x over previous
"""Trainium2 Bass kernel for nn_AELoss (MSE + smooth loss), 8-core data-parallel.

Strategy
--------
Shard batch dim (2048) across 8 cores -> 256 rows/core. Per core, 6 steps of
(b-group of 128, c); each step DMA-loads x,y tiles [128, t-chunk, 25] with
SWDGE f32->bf16 cast (HBM reads stay f32; all on-chip compute runs in bf16,
so DVE tensor_tensor hits its 2x perf mode).

Math: working in sum/difference space kills most of the work. With
d = x - y and p = x^2 - y^2 = (x+y)(x-y):
    s_in - s_tgt per (b,c,j) = sum_t d - sum_t p + p[0] - d[T-1]
    total[b,c] = sum_{j<J-1} |s_in - s_tgt|;  smooth = mean sqrt(total)/(J*T)
    mse = mean d^2
Per step the Vector engine does three full bf16 passes (s = x+y, d = x-y,
p = s*d in place over s) plus ONE combined binary fold tree over t for
(p, d) -> [128, 2, 25] f32 sums. The Scalar engine squares d with
accum_out for the per-partition MSE partial (junk elementwise output is
dumped into the consumed input tile). GpSimd only issues cast-DMAs and the
final partition_all_reduce -- any real GpSimd compute poisons DVE via the
shared SBUF port. Per-core partials are returned as a [1,2] tensor; the
host combines the 8 cores into the final scalar.

Measured on TRN2 (neuron-profile exec_time_ns): ~155-185 us vs a ~128 us
HBM roofline (46 MB of f32 reads/core at ~360 GB/s).
"""

import os
import sys

for _p in ("/opt/trn_rl_repo", "/root/.axon_site"):
    if os.path.isdir(_p) and _p not in sys.path:
        sys.path.insert(0, _p)

import numpy as np

import concourse.bass as bass
import concourse.tile as tile
from concourse import bacc, bass_isa, mybir
from concourse.bass_utils import run_bass_kernel_spmd

N_CORES = 8
B, C, T, J = 2048, 3, 300, 25
B_LOC = B // N_CORES          # 256 batch rows per core
P = 128                       # SBUF partitions
NG = B_LOC // P               # 2 b-groups per core
F32 = mybir.dt.float32
BF16 = mybir.dt.bfloat16
NSTEP = NG * C                # 6 (b-group, c) steps


def _fold_t2(nc, fs_pool, src, res):
    """Sum src [P, 2, 300, 25] over the t axis -> res [P, 2, 25] f32.

    Binary fold tree in bf16: 300 = 2*128 + 44, halve down to 2 rows,
    final add writes f32.
    """
    v = nc.vector
    fs = fs_pool.tile([P, 2, 128, J], BF16, tag="fold_bf")
    v.tensor_add(fs[:, :, 0:128, :], src[:, :, 0:128, :], src[:, :, 128:256, :])
    v.tensor_add(fs[:, :, 0:44, :], fs[:, :, 0:44, :], src[:, :, 256:300, :])
    n = 64
    while n >= 2:
        v.tensor_add(fs[:, :, 0:n, :], fs[:, :, 0:n, :], fs[:, :, n : 2 * n, :])
        n //= 2
    v.tensor_add(res[:, :, :], fs[:, :, 0, :], fs[:, :, 1, :])


def _body(tc, nc, x_d, y_d, out_d):
    cfg = CFG

    with (
        tc.tile_pool(name="inp", bufs=cfg["xy"]) as inp_pool,
        tc.tile_pool(name="sd", bufs=cfg["sd"]) as sd_pool,
        tc.tile_pool(name="fold", bufs=cfg["fold"]) as fold_pool,
        tc.tile_pool(name="small", bufs=3) as small_pool,
        tc.tile_pool(name="persist", bufs=1) as persist,
    ):
        totals6 = persist.tile([P, NSTEP], F32)       # per-step sum_j |s_in - s_tgt|
        mse14 = persist.tile([P, 2 * NSTEP + 2], F32)  # per-chunk sum (x-y)^2

        k = 0
        mcol = 0
        for g in range(NG):
            for c in range(C):
                # sd[:,0] = s = x+y (-> becomes p = x^2-y^2), sd[:,1] = d = x-y
                sd = sd_pool.tile([P, 2, T, J], BF16, tag="sd")
                # first step uses quarter chunks so compute starts sooner
                nch = 4 if k == 0 else 2
                tc_sz = T // nch
                for h in range(nch):
                    t0, t1 = h * tc_sz, (h + 1) * tc_sz
                    xyh = inp_pool.tile([P, 2, tc_sz, J], BF16, tag="xy")
                    nc.gpsimd.dma_start(
                        out=xyh[:, 0, :, :],
                        in_=x_d[g * P : (g + 1) * P, c, t0:t1, :],
                    )
                    nc.gpsimd.dma_start(
                        out=xyh[:, 1, :, :],
                        in_=y_d[g * P : (g + 1) * P, c, t0:t1, :],
                    )
                    nc.vector.tensor_add(
                        sd[:, 0, t0:t1, :], xyh[:, 0, :, :], xyh[:, 1, :, :]
                    )
                    nc.vector.tensor_sub(
                        sd[:, 1, t0:t1, :], xyh[:, 0, :, :], xyh[:, 1, :, :]
                    )
                    # p = s*d = x^2-y^2, in place over s
                    nc.vector.tensor_mul(
                        sd[:, 0, t0:t1, :], sd[:, 0, t0:t1, :], sd[:, 1, t0:t1, :]
                    )
                    # MSE partial for this chunk: sum d^2 (ACT square with
                    # accumulate; junk elementwise output goes to the xyh
                    # tile we just consumed)
                    nc.scalar.activation(
                        xyh[:, 0, :, :],
                        sd[:, 1, t0:t1, :],
                        mybir.ActivationFunctionType.Square,
                        accum_out=mse14[:, mcol : mcol + 1],
                    )
                    mcol += 1

                # one combined fold chain: res[:,0]=Pd=sum_t p, res[:,1]=Ad=sum_t d
                res = small_pool.tile([P, 2, J], F32, tag="res")
                _fold_t2(nc, fold_pool, sd, res)

                # D[j] = s_in - s_tgt = Ad - Pd + p[0] - d[T-1]
                D = small_pool.tile([P, J], F32, tag="D")
                nc.vector.tensor_sub(D[:, :], res[:, 1, :], res[:, 0, :])
                nc.vector.tensor_add(D[:, :], D[:, :], sd[:, 0, 0, :])
                nc.vector.tensor_sub(D[:, :], D[:, :], sd[:, 1, T - 1, :])
                nc.vector.reduce_sum(
                    totals6[:, k : k + 1],
                    D[:, 0 : J - 1],
                    axis=mybir.AxisListType.X,
                    apply_absolute_value=True,
                )

                k += 1

        # tail: sqrt(total)/(J*T) == sqrt(total * (1/(J*T))^2), summed over steps
        roots = small_pool.tile([P, NSTEP], F32, tag="roots")
        nc.scalar.activation(
            roots[:, :],
            totals6[:, :],
            mybir.ActivationFunctionType.Sqrt,
            scale=1.0 / float((J * T) ** 2),
        )
        final = small_pool.tile([P, 2], F32, tag="final")
        nc.vector.reduce_sum(final[:, 1:2], roots[:, :], axis=mybir.AxisListType.X)
        nc.vector.reduce_sum(final[:, 0:1], mse14[:, :], axis=mybir.AxisListType.X)

        red = small_pool.tile([P, 2], F32, tag="red")
        nc.gpsimd.partition_all_reduce(
            red[:, :], final[:, :], channels=P, reduce_op=bass_isa.ReduceOp.add
        )
        nc.sync.dma_start(out=out_d[0:1, :], in_=red[0:1, :])


_NC_CACHE = None
CFG = {"xy": 4, "sd": 3, "fold": 2}


def _build():
    global _NC_CACHE
    if _NC_CACHE is not None:
        return _NC_CACHE
    nc = bacc.Bacc("TRN2", target_bir_lowering=False, debug=False, num_devices=N_CORES)
    x_d = nc.dram_tensor("inputs", [B_LOC, C, T, J], F32, kind="ExternalInput")
    y_d = nc.dram_tensor("targets", [B_LOC, C, T, J], F32, kind="ExternalInput")
    out_d = nc.dram_tensor("out", [1, 2], F32, kind="ExternalOutput")
    with tile.TileContext(nc) as tc:
        _body(tc, nc, x_d.ap(), y_d.ap(), out_d.ap())
    nc.compile()
    _NC_CACHE = nc
    return nc


def _run(inputs, targets, trace=False, **kw):
    nc = _build()
    inputs = np.ascontiguousarray(inputs, dtype=np.float32)
    targets = np.ascontiguousarray(targets, dtype=np.float32)
    in_maps = [
        {
            "inputs": inputs[i * B_LOC : (i + 1) * B_LOC],
            "targets": targets[i * B_LOC : (i + 1) * B_LOC],
        }
        for i in range(N_CORES)
    ]
    res = run_bass_kernel_spmd(
        nc, in_maps, core_ids=list(range(N_CORES)), trace=trace, **kw
    )
    mse_sum = 0.0
    smooth_sum = 0.0
    for i in range(N_CORES):
        o = res.results[i]["out"]
        mse_sum += float(o[0, 0])
        smooth_sum += float(o[0, 1])
    value = 2.0 * (mse_sum / (B * C * T * J)) + 3.0 * (smooth_sum / (B * C))
    return np.array(value, dtype=np.float32), res


def kernel(inputs, targets):
    value, _ = _run(inputs, targets)
    return value
